# revision 8
# baseline (speedup 1.0000x reference)
import sys

sys.path.insert(0, "/opt/trn_rl_repo")

import numpy as np
import ml_dtypes
from contextlib import ExitStack

T, HID, FEAT, NACT = 128, 128, 512, 18
BS = 8
F = 16          # frames per pipeline chunk
NCH = T // F    # 8 chunks
NSWEEP = 8     # DEER fixed-point sweeps

_cache = {}


def _bf16(a):
    return np.asarray(a, dtype=np.float32).astype(ml_dtypes.bfloat16)


def _build_graph():
    from concourse import bacc, mybir, tile

    f32, bf16 = mybir.dt.float32, mybir.dt.bfloat16
    AF = mybir.ActivationFunctionType
    ALU = mybir.AluOpType

    nc = bacc.Bacc(None, target_bir_lowering=False)

    def din(name, shape, dt):
        return nc.declare_dram_parameter(name, shape, dt, isOutput=False)

    X = din("x", [T, 128, 441], bf16)
    W1 = din("w1", [128, 2, 32], bf16)
    B1 = din("b1", [32, 1], f32)
    W2 = din("w2", [128, 16, 64], bf16)
    B2 = din("b2", [64, 1], f32)
    W3 = din("w3", [64, 9, 128], bf16)
    B3 = din("b3", [128, 1], f32)
    FW1 = din("fw1", [128, 16, 128], bf16)   # (j*4+oc)
    FB1 = din("fb1", [128, 4], f32)
    WG = din("wg", [128, 12, 128], bf16)     # (gate*4+oc) lhsT chunks
    GB = din("gb", [128, 3], f32)
    UG = din("ug", [128, 3, 128], bf16)      # gate: Uz^T, Ur^T, Uh^T
    FW3 = din("fw3", [128, 18], bf16)
    FB3 = din("fb3", [18, 1], f32)
    OUT = nc.declare_dram_parameter("out", [T, NACT], f32, isOutput=True)
    HL = nc.declare_dram_parameter("hlast", [HID, 1], f32, isOutput=True)

    with tile.TileContext(nc) as tc, ExitStack() as ctx:
        consts = ctx.enter_context(tc.tile_pool(name="consts", bufs=1))

        w1sb = consts.tile([128, 2, 32], bf16)
        b1sb = consts.tile([32, 1], f32)
        w2sb = consts.tile([128, 16, 64], bf16)
        b2sb = consts.tile([64, 1], f32)
        w3sb = consts.tile([64, 9, 128], bf16)
        b3sb = consts.tile([128, 1], f32)
        fw1sb = consts.tile([128, 16, 128], bf16)
        fb1sb = consts.tile([128, 4], f32)
        wgsb = consts.tile([128, 12, 128], bf16)
        gbsb = consts.tile([128, 3], f32)
        ugsb = consts.tile([128, 3, 128], bf16)
        fw3sb = consts.tile([128, 18], bf16)
        fb3sb = consts.tile([18, 1], f32)
        for t_, d_ in [(w1sb, W1), (b1sb, B1), (w2sb, W2), (b2sb, B2),
                       (w3sb, W3), (b3sb, B3), (fw1sb, FW1), (fb1sb, FB1),
                       (wgsb, WG), (gbsb, GB), (ugsb, UG), (fw3sb, FW3),
                       (fb3sb, FB3)]:
            nc.sync.dma_start(t_[:], d_[:])

        # persistent activations
        a1A = consts.tile([32, F, 24, 24], bf16)
        a1B = consts.tile([32, F, 24, 24], bf16)
        c2A = [consts.tile([128, F, 6, 6], bf16, name=f"c2A{k}", tag=f"c2A{k}")
               for k in range(4)]
        c2B = [consts.tile([128, F, 6, 6], bf16, name=f"c2B{k}", tag=f"c2B{k}")
               for k in range(4)]
        f3 = consts.tile([128, T, 2, 2], bf16)
        fc1o = consts.tile([128, 4, T], bf16)
        wxsb = consts.tile([128, 3, T], f32)
        hA = consts.tile([128, T], bf16)
        hB = consts.tile([128, T], bf16)

        for t_ in (a1A, a1B):
            nc.vector.memset(t_[:, :, 0:2, :], 0.0)
            nc.vector.memset(t_[:, :, 22:24, :], 0.0)
            nc.vector.memset(t_[:, :, 2:22, 0:2], 0.0)
            nc.vector.memset(t_[:, :, 2:22, 22:24], 0.0)
        for t_ in c2A + c2B:
            nc.vector.memset(t_[:], 0.0)
        nc.vector.memset(hA[:], 0.0)
        nc.vector.memset(hB[:], 0.0)

        cctx = ExitStack()
        xp = cctx.enter_context(tc.tile_pool(name="xin", bufs=3))
        a2p = cctx.enter_context(tc.tile_pool(name="a2", bufs=2))
        ps1 = cctx.enter_context(tc.tile_pool(name="ps1", bufs=4, space="PSUM"))
        ps2 = cctx.enter_context(tc.tile_pool(name="ps2", bufs=2, space="PSUM"))
        ps3 = cctx.enter_context(tc.tile_pool(name="ps3", bufs=2, space="PSUM"))

        for ch in range(NCH):
            a1 = a1A if ch % 2 == 0 else a1B
            c2k = c2A if ch % 2 == 0 else c2B
            # ---- conv1 over 16 frames ----
            for g4 in range(4):
                f0 = ch * F + g4 * 4
                xt = xp.tile([128, 4, 441], bf16)
                nc.sync.dma_start(xt[:], X[f0:f0 + 4].rearrange("f p e -> p f e"))
                for fl in range(4):
                    ps = ps1.tile([32, 400], f32)
                    v = xt[:, fl, :].rearrange("p (i j) -> p i j", i=21)
                    for di in range(2):
                        nc.tensor.matmul(
                            ps[:], w1sb[:, di, :], v[:, di:di + 20, 0:20],
                            start=(di == 0), stop=(di == 1))
                    fidx = g4 * 4 + fl
                    dst = a1[:, fidx, 2:22, 2:22]
                    pv = ps[:].rearrange("p (i j) -> p i j", i=20)
                    if fl % 2 == 0:
                        nc.vector.tensor_scalar(dst, pv, b1sb[:], 0.0,
                                                ALU.add, ALU.max)
                    else:
                        nc.scalar.activation(dst, pv, AF.Relu, bias=b1sb[:])
            # ---- conv2 input s2d repack (16 DVE copies) ----
            for k in range(4):
                for rx in range(4):
                    y0 = 1 if k < 2 else 0
                    x0 = 1 if rx < 2 else 0
                    ny, nx = 6 - y0, 6 - x0
                    # stepped view via rearrange: rows 4Y+k, cols 4X+rx
                    av = a1[:, :, :, :].rearrange("p f (Y s) (Xx t) -> p f s Y t Xx",
                                                  s=4, t=4)
                    src = av[:, :, k, y0:y0 + ny, rx, x0:x0 + nx]
                    dst = c2k[k][32 * rx:32 * rx + 32, :, y0:y0 + ny, x0:x0 + nx]
                    nc.gpsimd.tensor_copy(dst, src)
            # ---- conv2 ----
            p2 = ps2.tile([64, F, 5, 5], f32)
            idx = 0
            for dq in range(4):
                di, dj = dq >> 1, dq & 1
                for k in range(4):
                    rhs = c2k[k][:, :, di:di + 5, dj:dj + 5]
                    nc.tensor.matmul(p2[:], w2sb[:, dq * 4 + k, :], rhs,
                                     start=(idx == 0), stop=(idx == 15))
                    idx += 1
            a2 = a2p.tile([64, F, 6, 6], bf16)
            nc.scalar.activation(a2[:, :, 0:5, 0:5], p2[:], AF.Relu, bias=b2sb[:])
            # ---- conv3 ----
            p3 = ps3.tile([128, F, 2, 2], f32)
            a2v = a2[:, :, :, :].rearrange("p f (Y s) (Xx t) -> p s t f Y Xx",
                                           s=2, t=2)
            idx = 0
            for ky in range(3):
                for kx in range(3):
                    rhs = a2v[:, ky % 2, kx % 2, :, ky // 2:ky // 2 + 2,
                              kx // 2:kx // 2 + 2]
                    nc.tensor.matmul(p3[:], w3sb[:, idx, :], rhs,
                                     start=(idx == 0), stop=(idx == 8))
                    idx += 1
            nc.scalar.activation(f3[:, ch * F:(ch + 1) * F, :, :], p3[:],
                                 AF.Relu, bias=b3sb[:])

        # ---- fc1 (all T at once) ----
        cctx.close()
        psg = ctx.enter_context(tc.tile_pool(name="psg", bufs=4, space="PSUM"))
        f3v = f3[:, :, :, :].rearrange("p t a b -> p (a b) t")
        for oc in range(4):
            pf = psg.tile([128, T], f32, tag="g", name="pf")
            for j in range(4):
                nc.tensor.matmul(pf[:], fw1sb[:, j * 4 + oc, :], f3v[:, j, :],
                                 start=(j == 0), stop=(j == 3))
            nc.scalar.activation(fc1o[:, oc, :], pf[:], AF.Relu,
                                 bias=fb1sb[:, oc:oc + 1])
        # ---- gate input projections wx ----
        for gi in range(3):
            px = psg.tile([128, T], f32, tag="g", name="px")
            for oc in range(4):
                nc.tensor.matmul(px[:], wgsb[:, gi * 4 + oc, :], fc1o[:, oc, :],
                                 start=(oc == 0), stop=(oc == 3))
            nc.scalar.activation(wxsb[:, gi, :], px[:], AF.Identity,
                                 bias=gbsb[:, gi:gi + 1])

        # ---- DEER fixed-point GRU ----
        dp = ctx.enter_context(tc.tile_pool(name="deer", bufs=2))
        hs_final = None
        for s in range(NSWEEP):
            hp = hA if s % 2 == 0 else hB
            hn = hB if s % 2 == 0 else hA
            pz = psg.tile([128, T], f32, tag="g", name="pz")
            pr = psg.tile([128, T], f32, tag="g", name="pr")
            nc.tensor.matmul(pz[:], ugsb[:, 0, :], hp[:], start=True, stop=True)
            nc.tensor.matmul(pr[:], ugsb[:, 1, :], hp[:], start=True, stop=True)
            tz = dp.tile([128, T], f32, tag="tz")
            tr = dp.tile([128, T], f32, tag="tr")
            nc.vector.tensor_tensor(tz[:], pz[:], wxsb[:, 0, :], ALU.add)
            nc.vector.tensor_tensor(tr[:], pr[:], wxsb[:, 1, :], ALU.add)
            Z = dp.tile([128, T], f32, tag="Z")
            R = dp.tile([128, T], f32, tag="R")
            nc.scalar.activation(Z[:], tz[:], AF.Sigmoid)
            nc.scalar.activation(R[:], tr[:], AF.Sigmoid)
            RH = dp.tile([128, T], bf16, tag="RH")
            nc.vector.tensor_tensor(RH[:], R[:], hp[:], ALU.mult)
            ph = psg.tile([128, T], f32, tag="g", name="ph")
            nc.tensor.matmul(ph[:], ugsb[:, 2, :], RH[:], start=True, stop=True)
            th = dp.tile([128, T], f32, tag="th")
            nc.vector.tensor_tensor(th[:], ph[:], wxsb[:, 2, :], ALU.add)
            HH = dp.tile([128, T], f32, tag="HH")
            nc.scalar.activation(HH[:], th[:], AF.Tanh)
            A = dp.tile([128, T], f32, tag="A")
            nc.vector.tensor_scalar(A[:], Z[:], -1.0, 1.0, ALU.mult, ALU.add)
            Bt = dp.tile([128, T], f32, tag="B")
            nc.vector.tensor_tensor(Bt[:], Z[:], HH[:], ALU.mult)
            hs = dp.tile([128, T], bf16, tag="hs")
            nc.vector.tensor_tensor_scan(hs[:], A[:], Bt[:], 0.0,
                                         ALU.mult, ALU.add)
            if s < NSWEEP - 1:
                nc.vector.tensor_copy(hn[:, 1:T], hs[:, 0:T - 1])
            hs_final = hs

        # ---- fc3 + outputs ----
        po = psg.tile([18, T], f32, tag="g", name="po")
        nc.tensor.matmul(po[:], fw3sb[:], hs_final[:], start=True, stop=True)
        osb = dp.tile([18, T], f32, tag="osb")
        nc.scalar.activation(osb[:], po[:], AF.Identity, bias=fb3sb[:])
        nc.sync.dma_start(OUT[:, :].rearrange("t a -> a t"), osb[:])
        hl32 = dp.tile([128, 1], f32, tag="hl32")
        nc.vector.tensor_copy(hl32[:], hs_final[:, T - 1:T])
        nc.sync.dma_start(HL[:, :], hl32[:])

    nc.compile()
    return nc


def _prep_shared(inputs):
    """Host-side weight preprocessing (shared across cores)."""
    g = {k: np.asarray(v, dtype=np.float32) for k, v in inputs.items()}
    d = {}
    # conv1: lhsT halves [128, 2, 32]; p<64: (r,c); p>=64: (r, 8+c); per di
    w1 = g["conv1_w"]  # [32,1,16,16]
    w1s = np.zeros((128, 2, 32), np.float32)
    for di in range(2):
        for p in range(64):
            r, c = p // 8, p % 8
            w1s[p, di] = w1[:, 0, di * 8 + r, c]
            w1s[64 + p, di] = w1[:, 0, di * 8 + r, 8 + c]
    d["w1"] = _bf16(w1s)
    d["b1"] = g["conv1_b"].reshape(32, 1)
    # conv2 chunks: (dq=(di,dj), k): [p=32rx+c, o] = w2[o, c, 4di+k, 4dj+rx]
    w2 = g["conv2_w"]  # [64,32,8,8]
    w2s = np.zeros((128, 16, 64), np.float32)
    for dq in range(4):
        di, dj = dq >> 1, dq & 1
        for k in range(4):
            for rx in range(4):
                for c in range(32):
                    w2s[32 * rx + c, dq * 4 + k] = w2[:, c, 4 * di + k,
                                                      4 * dj + rx]
    d["w2"] = _bf16(w2s)
    d["b2"] = g["conv2_b"].reshape(64, 1)
    # conv3: [64, 9, 128]: lhsT = w3[:,:,ky,kx].T
    w3 = g["conv3_w"]  # [128,64,3,3]
    w3s = np.zeros((64, 9, 128), np.float32)
    for ky in range(3):
        for kx in range(3):
            w3s[:, ky * 3 + kx] = w3[:, :, ky, kx].T
    d["w3"] = _bf16(w3s)
    d["b3"] = g["conv3_b"].reshape(128, 1)
    # fc1 chunks (j, oc): fc1_w[oc*128:(oc+1)*128, j::4].T  (feat idx = c*4 + pos)
    fw = g["fc1_w"]  # [512, 512]
    fw1 = np.zeros((128, 16, 128), np.float32)
    for j in range(4):
        for oc in range(4):
            fw1[:, j * 4 + oc] = fw[oc * 128:(oc + 1) * 128, j::4].T
    d["fw1"] = _bf16(fw1)
    d["fb1"] = g["fc1_b"].reshape(128, 4, order="F").copy()
    # gates W chunks: (gate, oc): W[:, oc*128:(oc+1)*128].T over fc1o oc blocks
    wg = np.zeros((128, 12, 128), np.float32)
    for gi, Wm in enumerate([g["Wz"], g["Wr"], g["Wh"]]):
        for oc in range(4):
            wg[:, gi * 4 + oc] = Wm[:, oc * 128:(oc + 1) * 128].T
    d["wg"] = _bf16(wg)
    d["gb"] = np.stack([g["bz"], g["br"], g["bh"]], axis=1)
    ug = np.stack([g["Uz"].T, g["Ur"].T, g["Uh"].T], axis=1)
    d["ug"] = _bf16(ug)
    d["fw3"] = _bf16(g["fc3_w"].T)  # [128, 18]
    d["fb3"] = g["fc3_b"].reshape(18, 1)
    return d


def _prep_x(xb):
    """[T,160,160] f32 -> s2d bf16 [T, 128, 441] (half2 = J+1 shifted)."""
    xpad = np.zeros((T, 168, 176), np.float32)
    xpad[:, 4:164, 4:164] = xb
    blk = xpad.reshape(T, 21, 8, 22, 8)
    h1 = blk[:, :, :, 0:21].transpose(0, 2, 4, 1, 3).reshape(T, 64, 441)
    h2 = blk[:, :, :, 1:22].transpose(0, 2, 4, 1, 3).reshape(T, 64, 441)
    return _bf16(np.concatenate([h1, h2], axis=1))


def kernel(**inputs):
    from concourse.bass_utils import run_bass_kernel_spmd

    if "nc" not in _cache:
        _cache["nc"] = _build_graph()
    nc = _cache["nc"]

    shared = _prep_shared(inputs)
    x = np.asarray(inputs["x"], dtype=np.float32)  # [8,128,160,160,1]
    in_maps = []
    for b in range(BS):
        m = dict(shared)
        m["x"] = _prep_x(x[b, :, :, :, 0])
        in_maps.append(m)

    import os
    trace = bool(os.environ.get("BASS_KERNEL_TRACE"))
    if trace:
        import types
        if "antenv.axon_hooks" not in sys.modules:
            import antenv
            mod = types.ModuleType("antenv.axon_hooks")
            holder = {"hook": None}
            mod.set_axon_ntff_profile_hook = \
                lambda h: holder.__setitem__("hook", h)
            mod.get_axon_ntff_profile_hook = lambda: holder["hook"]
            sys.modules["antenv.axon_hooks"] = mod
            antenv.axon_hooks = mod
            try:
                from trn_agent_boot.trn_boot import _ntff_profile_via_ctypes
                mod.set_axon_ntff_profile_hook(
                    _ntff_profile_via_ctypes("/opt/axon/libaxon_pjrt.so"))
            except Exception as e:
                print("ntff hook registration failed:", e)
    res = run_bass_kernel_spmd(nc, in_maps, core_ids=list(range(BS)),
                               trace=trace)
    if trace:
        _cache["exec_time_ns"] = res.exec_time_ns
        _cache["profile_json"] = res.profile_json
    outs = res.results
    out = np.stack([outs[b]["out"] for b in range(BS)], axis=0)
    hl = np.stack([outs[b]["hlast"][:, 0] for b in range(BS)], axis=0)
    return out.astype(np.float32), hl.astype(np.float32)[None]


# revision 9
# speedup vs baseline: 1.7350x; 1.7350x over previous
import sys

sys.path.insert(0, "/opt/trn_rl_repo")

import numpy as np
import ml_dtypes
from contextlib import ExitStack

T, HID, FEAT, NACT = 128, 128, 512, 18
BS = 8
F = 16          # frames per pipeline chunk
NCH = T // F    # 8 chunks
NSWEEP = 8     # DEER fixed-point sweeps

_cache = {}


def _bf16(a):
    return np.asarray(a, dtype=np.float32).astype(ml_dtypes.bfloat16)


def _build_graph():
    from concourse import bacc, mybir, tile

    f32, bf16 = mybir.dt.float32, mybir.dt.bfloat16
    AF = mybir.ActivationFunctionType
    ALU = mybir.AluOpType

    nc = bacc.Bacc(None, target_bir_lowering=False)

    def din(name, shape, dt):
        return nc.declare_dram_parameter(name, shape, dt, isOutput=False)

    X = din("x", [T, 128, 441], bf16)
    W1 = din("w1", [128, 2, 32], bf16)
    B1 = din("b1", [32, 1], f32)
    W2 = din("w2", [128, 16, 64], bf16)
    B2 = din("b2", [64, 1], f32)
    W3 = din("w3", [64, 9, 128], bf16)
    B3 = din("b3", [128, 1], f32)
    FW1 = din("fw1", [128, 16, 128], bf16)   # (j*4+oc)
    FB1 = din("fb1", [128, 4], f32)
    WG = din("wg", [128, 12, 128], bf16)     # (gate*4+oc) lhsT chunks
    GB = din("gb", [128, 3], f32)
    UG = din("ug", [128, 3, 128], bf16)      # gate: Uz^T, Ur^T, Uh^T
    FW3 = din("fw3", [128, 18], bf16)
    FB3 = din("fb3", [18, 1], f32)
    OUT = nc.declare_dram_parameter("out", [T, NACT], f32, isOutput=True)
    HL = nc.declare_dram_parameter("hlast", [HID, 1], f32, isOutput=True)

    with tile.TileContext(nc) as tc, ExitStack() as ctx:
        consts = ctx.enter_context(tc.tile_pool(name="consts", bufs=1))

        w1sb = consts.tile([128, 2, 32], bf16)
        b1sb = consts.tile([32, 1], f32)
        w2sb = consts.tile([128, 16, 64], bf16)
        b2sb = consts.tile([64, 1], f32)
        w3sb = consts.tile([64, 9, 128], bf16)
        b3sb = consts.tile([128, 1], f32)
        fw1sb = consts.tile([128, 16, 128], bf16)
        fb1sb = consts.tile([128, 4], f32)
        wgsb = consts.tile([128, 12, 128], bf16)
        gbsb = consts.tile([128, 3], f32)
        ugsb = consts.tile([128, 3, 128], bf16)
        fw3sb = consts.tile([128, 18], bf16)
        fb3sb = consts.tile([18, 1], f32)
        for t_, d_ in [(w1sb, W1), (b1sb, B1), (w2sb, W2), (b2sb, B2),
                       (w3sb, W3), (b3sb, B3), (fw1sb, FW1), (fb1sb, FB1),
                       (wgsb, WG), (gbsb, GB), (ugsb, UG), (fw3sb, FW3),
                       (fb3sb, FB3)]:
            nc.sync.dma_start(t_[:], d_[:])

        # persistent activations
        a1A = consts.tile([32, F, 24, 24], bf16)
        a1B = consts.tile([32, F, 24, 24], bf16)
        c2A = [consts.tile([128, F, 6, 6], bf16, name=f"c2A{k}", tag=f"c2A{k}")
               for k in range(4)]
        c2B = [consts.tile([128, F, 6, 6], bf16, name=f"c2B{k}", tag=f"c2B{k}")
               for k in range(4)]
        f3 = consts.tile([128, T, 2, 2], bf16)
        fc1o = consts.tile([128, 4, T], bf16)
        wxsb = consts.tile([128, 3, T], f32)
        hA = consts.tile([128, T], bf16)
        hB = consts.tile([128, T], bf16)

        for t_ in (a1A, a1B):
            nc.vector.memset(t_[:, :, 0:2, :], 0.0)
            nc.vector.memset(t_[:, :, 22:24, :], 0.0)
            nc.vector.memset(t_[:, :, 2:22, 0:2], 0.0)
            nc.vector.memset(t_[:, :, 2:22, 22:24], 0.0)
        for t_ in c2A + c2B:
            nc.vector.memset(t_[:], 0.0)
        nc.vector.memset(hA[:], 0.0)
        nc.vector.memset(hB[:], 0.0)

        cctx = ExitStack()
        xp = cctx.enter_context(tc.tile_pool(name="xin", bufs=3))
        a2p = cctx.enter_context(tc.tile_pool(name="a2", bufs=2))
        ps1 = cctx.enter_context(tc.tile_pool(name="ps1", bufs=4, space="PSUM"))
        ps2 = cctx.enter_context(tc.tile_pool(name="ps2", bufs=2, space="PSUM"))
        ps3 = cctx.enter_context(tc.tile_pool(name="ps3", bufs=2, space="PSUM"))

        for ch in range(NCH):
            a1 = a1A if ch % 2 == 0 else a1B
            c2k = c2A if ch % 2 == 0 else c2B
            # ---- conv1 over 16 frames ----
            for g4 in range(4):
                f0 = ch * F + g4 * 4
                xt = xp.tile([128, 4, 441], bf16)
                nc.sync.dma_start(xt[:], X[f0:f0 + 4].rearrange("f p e -> p f e"))
                for fl in range(4):
                    ps = ps1.tile([32, 400], f32)
                    v = xt[:, fl, :].rearrange("p (i j) -> p i j", i=21)
                    for di in range(2):
                        nc.tensor.matmul(
                            ps[:], w1sb[:, di, :], v[:, di:di + 20, 0:20],
                            start=(di == 0), stop=(di == 1))
                    fidx = g4 * 4 + fl
                    dst = a1[:, fidx, 2:22, 2:22]
                    pv = ps[:].rearrange("p (i j) -> p i j", i=20)
                    if fl % 2 == 0:
                        nc.vector.tensor_scalar(dst, pv, b1sb[:], 0.0,
                                                ALU.add, ALU.max)
                    else:
                        nc.scalar.activation(dst, pv, AF.Relu, bias=b1sb[:])
            # ---- conv2 input s2d repack (16 DVE copies) ----
            for k in range(4):
                for rx in range(4):
                    y0 = 1 if k < 2 else 0
                    x0 = 1 if rx < 2 else 0
                    ny, nx = 6 - y0, 6 - x0
                    # stepped view via rearrange: rows 4Y+k, cols 4X+rx
                    av = a1[:, :, :, :].rearrange("p f (Y s) (Xx t) -> p f s Y t Xx",
                                                  s=4, t=4)
                    src = av[:, :, k, y0:y0 + ny, rx, x0:x0 + nx]
                    dst = c2k[k][32 * rx:32 * rx + 32, :, y0:y0 + ny, x0:x0 + nx]
                    if rx % 2 == 0:
                        nc.vector.tensor_copy(dst, src)
                    else:
                        nc.scalar.activation(dst, src, AF.Copy)
            # ---- conv2 ----
            p2 = ps2.tile([64, F, 5, 5], f32)
            idx = 0
            for dq in range(4):
                di, dj = dq >> 1, dq & 1
                for k in range(4):
                    rhs = c2k[k][:, :, di:di + 5, dj:dj + 5]
                    nc.tensor.matmul(p2[:], w2sb[:, dq * 4 + k, :], rhs,
                                     start=(idx == 0), stop=(idx == 15))
                    idx += 1
            a2 = a2p.tile([64, F, 6, 6], bf16)
            nc.scalar.activation(a2[:, :, 0:5, 0:5], p2[:], AF.Relu, bias=b2sb[:])
            # ---- conv3 ----
            p3 = ps3.tile([128, F, 2, 2], f32)
            a2v = a2[:, :, :, :].rearrange("p f (Y s) (Xx t) -> p s t f Y Xx",
                                           s=2, t=2)
            idx = 0
            for ky in range(3):
                for kx in range(3):
                    rhs = a2v[:, ky % 2, kx % 2, :, ky // 2:ky // 2 + 2,
                              kx // 2:kx // 2 + 2]
                    nc.tensor.matmul(p3[:], w3sb[:, idx, :], rhs,
                                     start=(idx == 0), stop=(idx == 8))
                    idx += 1
            nc.scalar.activation(f3[:, ch * F:(ch + 1) * F, :, :], p3[:],
                                 AF.Relu, bias=b3sb[:])

        # ---- fc1 (all T at once) ----
        cctx.close()
        psg = ctx.enter_context(tc.tile_pool(name="psg", bufs=4, space="PSUM"))
        f3v = f3[:, :, :, :].rearrange("p t a b -> p (a b) t")
        for oc in range(4):
            pf = psg.tile([128, T], f32, tag="g", name="pf")
            for j in range(4):
                nc.tensor.matmul(pf[:], fw1sb[:, j * 4 + oc, :], f3v[:, j, :],
                                 start=(j == 0), stop=(j == 3))
            nc.scalar.activation(fc1o[:, oc, :], pf[:], AF.Relu,
                                 bias=fb1sb[:, oc:oc + 1])
        # ---- gate input projections wx ----
        for gi in range(3):
            px = psg.tile([128, T], f32, tag="g", name="px")
            for oc in range(4):
                nc.tensor.matmul(px[:], wgsb[:, gi * 4 + oc, :], fc1o[:, oc, :],
                                 start=(oc == 0), stop=(oc == 3))
            nc.scalar.activation(wxsb[:, gi, :], px[:], AF.Identity,
                                 bias=gbsb[:, gi:gi + 1])

        # ---- DEER fixed-point GRU ----
        dp = ctx.enter_context(tc.tile_pool(name="deer", bufs=2))
        hs_final = None
        for s in range(NSWEEP):
            hp = hA if s % 2 == 0 else hB
            hn = hB if s % 2 == 0 else hA
            pz = psg.tile([128, T], f32, tag="g", name="pz")
            pr = psg.tile([128, T], f32, tag="g", name="pr")
            nc.tensor.matmul(pz[:], ugsb[:, 0, :], hp[:], start=True, stop=True)
            nc.tensor.matmul(pr[:], ugsb[:, 1, :], hp[:], start=True, stop=True)
            tz = dp.tile([128, T], f32, tag="tz")
            tr = dp.tile([128, T], f32, tag="tr")
            nc.vector.tensor_tensor(tz[:], pz[:], wxsb[:, 0, :], ALU.add)
            nc.vector.tensor_tensor(tr[:], pr[:], wxsb[:, 1, :], ALU.add)
            Z = dp.tile([128, T], f32, tag="Z")
            R = dp.tile([128, T], f32, tag="R")
            nc.scalar.activation(Z[:], tz[:], AF.Sigmoid)
            nc.scalar.activation(R[:], tr[:], AF.Sigmoid)
            RH = dp.tile([128, T], bf16, tag="RH")
            nc.vector.tensor_tensor(RH[:], R[:], hp[:], ALU.mult)
            ph = psg.tile([128, T], f32, tag="g", name="ph")
            nc.tensor.matmul(ph[:], ugsb[:, 2, :], RH[:], start=True, stop=True)
            th = dp.tile([128, T], f32, tag="th")
            nc.vector.tensor_tensor(th[:], ph[:], wxsb[:, 2, :], ALU.add)
            HH = dp.tile([128, T], f32, tag="HH")
            nc.scalar.activation(HH[:], th[:], AF.Tanh)
            A = dp.tile([128, T], f32, tag="A")
            nc.vector.tensor_scalar(A[:], Z[:], -1.0, 1.0, ALU.mult, ALU.add)
            Bt = dp.tile([128, T], f32, tag="B")
            nc.vector.tensor_tensor(Bt[:], Z[:], HH[:], ALU.mult)
            hs = dp.tile([128, T], bf16, tag="hs")
            nc.vector.tensor_tensor_scan(hs[:], A[:], Bt[:], 0.0,
                                         ALU.mult, ALU.add)
            if s < NSWEEP - 1:
                nc.vector.tensor_copy(hn[:, 1:T], hs[:, 0:T - 1])
            hs_final = hs

        # ---- fc3 + outputs ----
        po = psg.tile([18, T], f32, tag="g", name="po")
        nc.tensor.matmul(po[:], fw3sb[:], hs_final[:], start=True, stop=True)
        osb = dp.tile([18, T], f32, tag="osb")
        nc.scalar.activation(osb[:], po[:], AF.Identity, bias=fb3sb[:])
        nc.sync.dma_start(OUT[:, :].rearrange("t a -> a t"), osb[:])
        hl32 = dp.tile([128, 1], f32, tag="hl32")
        nc.vector.tensor_copy(hl32[:], hs_final[:, T - 1:T])
        nc.sync.dma_start(HL[:, :], hl32[:])

    nc.compile()
    return nc


def _prep_shared(inputs):
    """Host-side weight preprocessing (shared across cores)."""
    g = {k: np.asarray(v, dtype=np.float32) for k, v in inputs.items()}
    d = {}
    # conv1: lhsT halves [128, 2, 32]; p<64: (r,c); p>=64: (r, 8+c); per di
    w1 = g["conv1_w"]  # [32,1,16,16]
    w1s = np.zeros((128, 2, 32), np.float32)
    for di in range(2):
        for p in range(64):
            r, c = p // 8, p % 8
            w1s[p, di] = w1[:, 0, di * 8 + r, c]
            w1s[64 + p, di] = w1[:, 0, di * 8 + r, 8 + c]
    d["w1"] = _bf16(w1s)
    d["b1"] = g["conv1_b"].reshape(32, 1)
    # conv2 chunks: (dq=(di,dj), k): [p=32rx+c, o] = w2[o, c, 4di+k, 4dj+rx]
    w2 = g["conv2_w"]  # [64,32,8,8]
    w2s = np.zeros((128, 16, 64), np.float32)
    for dq in range(4):
        di, dj = dq >> 1, dq & 1
        for k in range(4):
            for rx in range(4):
                for c in range(32):
                    w2s[32 * rx + c, dq * 4 + k] = w2[:, c, 4 * di + k,
                                                      4 * dj + rx]
    d["w2"] = _bf16(w2s)
    d["b2"] = g["conv2_b"].reshape(64, 1)
    # conv3: [64, 9, 128]: lhsT = w3[:,:,ky,kx].T
    w3 = g["conv3_w"]  # [128,64,3,3]
    w3s = np.zeros((64, 9, 128), np.float32)
    for ky in range(3):
        for kx in range(3):
            w3s[:, ky * 3 + kx] = w3[:, :, ky, kx].T
    d["w3"] = _bf16(w3s)
    d["b3"] = g["conv3_b"].reshape(128, 1)
    # fc1 chunks (j, oc): fc1_w[oc*128:(oc+1)*128, j::4].T  (feat idx = c*4 + pos)
    fw = g["fc1_w"]  # [512, 512]
    fw1 = np.zeros((128, 16, 128), np.float32)
    for j in range(4):
        for oc in range(4):
            fw1[:, j * 4 + oc] = fw[oc * 128:(oc + 1) * 128, j::4].T
    d["fw1"] = _bf16(fw1)
    d["fb1"] = g["fc1_b"].reshape(128, 4, order="F").copy()
    # gates W chunks: (gate, oc): W[:, oc*128:(oc+1)*128].T over fc1o oc blocks
    wg = np.zeros((128, 12, 128), np.float32)
    for gi, Wm in enumerate([g["Wz"], g["Wr"], g["Wh"]]):
        for oc in range(4):
            wg[:, gi * 4 + oc] = Wm[:, oc * 128:(oc + 1) * 128].T
    d["wg"] = _bf16(wg)
    d["gb"] = np.stack([g["bz"], g["br"], g["bh"]], axis=1)
    ug = np.stack([g["Uz"].T, g["Ur"].T, g["Uh"].T], axis=1)
    d["ug"] = _bf16(ug)
    d["fw3"] = _bf16(g["fc3_w"].T)  # [128, 18]
    d["fb3"] = g["fc3_b"].reshape(18, 1)
    return d


def _prep_x(xb):
    """[T,160,160] f32 -> s2d bf16 [T, 128, 441] (half2 = J+1 shifted)."""
    xpad = np.zeros((T, 168, 176), np.float32)
    xpad[:, 4:164, 4:164] = xb
    blk = xpad.reshape(T, 21, 8, 22, 8)
    h1 = blk[:, :, :, 0:21].transpose(0, 2, 4, 1, 3).reshape(T, 64, 441)
    h2 = blk[:, :, :, 1:22].transpose(0, 2, 4, 1, 3).reshape(T, 64, 441)
    return _bf16(np.concatenate([h1, h2], axis=1))


def kernel(**inputs):
    from concourse.bass_utils import run_bass_kernel_spmd

    if "nc" not in _cache:
        _cache["nc"] = _build_graph()
    nc = _cache["nc"]

    shared = _prep_shared(inputs)
    x = np.asarray(inputs["x"], dtype=np.float32)  # [8,128,160,160,1]
    in_maps = []
    for b in range(BS):
        m = dict(shared)
        m["x"] = _prep_x(x[b, :, :, :, 0])
        in_maps.append(m)

    import os
    trace = bool(os.environ.get("BASS_KERNEL_TRACE"))
    if trace:
        import types
        if "antenv.axon_hooks" not in sys.modules:
            import antenv
            mod = types.ModuleType("antenv.axon_hooks")
            holder = {"hook": None}
            mod.set_axon_ntff_profile_hook = \
                lambda h: holder.__setitem__("hook", h)
            mod.get_axon_ntff_profile_hook = lambda: holder["hook"]
            sys.modules["antenv.axon_hooks"] = mod
            antenv.axon_hooks = mod
            try:
                from trn_agent_boot.trn_boot import _ntff_profile_via_ctypes
                mod.set_axon_ntff_profile_hook(
                    _ntff_profile_via_ctypes("/opt/axon/libaxon_pjrt.so"))
            except Exception as e:
                print("ntff hook registration failed:", e)
    res = run_bass_kernel_spmd(nc, in_maps, core_ids=list(range(BS)),
                               trace=trace)
    if trace:
        _cache["exec_time_ns"] = res.exec_time_ns
        _cache["profile_json"] = res.profile_json
    outs = res.results
    out = np.stack([outs[b]["out"] for b in range(BS)], axis=0)
    hl = np.stack([outs[b]["hlast"][:, 0] for b in range(BS)], axis=0)
    return out.astype(np.float32), hl.astype(np.float32)[None]


# revision 10
# speedup vs baseline: 2.5796x; 1.4868x over previous
import sys

sys.path.insert(0, "/opt/trn_rl_repo")

import numpy as np
import ml_dtypes
from contextlib import ExitStack

T, HID, FEAT, NACT = 128, 128, 512, 18
BS = 8
F = 8           # frames per pipeline chunk
NCH = T // F    # 8 chunks
NSWEEP = 6     # DEER fixed-point sweeps

_cache = {}


def _bf16(a):
    return np.asarray(a, dtype=np.float32).astype(ml_dtypes.bfloat16)


def _build_graph():
    from concourse import bacc, mybir, tile

    f32, bf16 = mybir.dt.float32, mybir.dt.bfloat16
    AF = mybir.ActivationFunctionType
    ALU = mybir.AluOpType

    nc = bacc.Bacc(None, target_bir_lowering=False)

    def din(name, shape, dt):
        return nc.declare_dram_parameter(name, shape, dt, isOutput=False)

    X = din("x", [T, 128, 441], bf16)
    W1 = din("w1", [128, 2, 32], bf16)
    B1 = din("b1", [128, 1], f32)
    W2 = din("w2", [128, 16, 64], bf16)
    B2 = din("b2", [64, 1], f32)
    W3 = din("w3", [64, 9, 128], bf16)
    B3 = din("b3", [128, 1], f32)
    FW1 = din("fw1", [128, 16, 128], bf16)   # (j*4+oc)
    FB1 = din("fb1", [128, 4], f32)
    WG = din("wg", [128, 12, 128], bf16)     # (gate*4+oc) lhsT chunks
    GB = din("gb", [128, 3], f32)
    UG = din("ug", [128, 3, 128], bf16)      # gate: Uz^T, Ur^T, Uh^T
    FW3 = din("fw3", [128, 18], bf16)
    FB3 = din("fb3", [18, 1], f32)
    OUT = nc.declare_dram_parameter("out", [T, NACT], f32, isOutput=True)
    HL = nc.declare_dram_parameter("hlast", [HID, 1], f32, isOutput=True)

    with tile.TileContext(nc) as tc, ExitStack() as ctx:
        consts = ctx.enter_context(tc.tile_pool(name="consts", bufs=1))

        w1sb = consts.tile([128, 2, 32], bf16)
        b1sb = consts.tile([128, 1], f32)
        w2sb = consts.tile([128, 16, 64], bf16)
        b2sb = consts.tile([64, 1], f32)
        w3sb = consts.tile([64, 9, 128], bf16)
        b3sb = consts.tile([128, 1], f32)
        fw1sb = consts.tile([128, 16, 128], bf16)
        fb1sb = consts.tile([128, 4], f32)
        wgsb = consts.tile([128, 12, 128], bf16)
        gbsb = consts.tile([128, 3], f32)
        ugsb = consts.tile([128, 3, 128], bf16)
        fw3sb = consts.tile([128, 18], bf16)
        fb3sb = consts.tile([18, 1], f32)
        for t_, d_ in [(w1sb, W1), (b1sb, B1), (w2sb, W2), (b2sb, B2),
                       (w3sb, W3), (b3sb, B3), (fw1sb, FW1), (fb1sb, FB1),
                       (wgsb, WG), (gbsb, GB), (ugsb, UG), (fw3sb, FW3),
                       (fb3sb, FB3)]:
            nc.sync.dma_start(t_[:], d_[:])

        # persistent activations
        c2A = [consts.tile([128, F, 6, 6], bf16, name=f"c2A{k}", tag=f"c2A{k}")
               for k in range(4)]
        c2B = [consts.tile([128, F, 6, 6], bf16, name=f"c2B{k}", tag=f"c2B{k}")
               for k in range(4)]
        a2A = consts.tile([64, 16, 6, 6], bf16)
        a2B = consts.tile([64, 16, 6, 6], bf16)
        f3 = consts.tile([128, T, 2, 2], bf16)
        fc1o = consts.tile([128, 4, T], bf16)
        wxsb = consts.tile([128, 3, T], f32)
        hA = consts.tile([128, T], bf16)
        hB = consts.tile([128, T], bf16)

        for t_ in c2A + c2B:
            nc.vector.memset(t_[:], 0.0)
        nc.vector.memset(hA[:], 0.0)
        nc.vector.memset(hB[:], 0.0)

        cctx = ExitStack()
        xp = cctx.enter_context(tc.tile_pool(name="xin", bufs=3))
        ps1 = cctx.enter_context(tc.tile_pool(name="ps1", bufs=4, space="PSUM"))
        ps2 = cctx.enter_context(tc.tile_pool(name="ps2", bufs=2, space="PSUM"))
        ps3 = cctx.enter_context(tc.tile_pool(name="ps3", bufs=2, space="PSUM"))

        for ch in range(NCH):
            c2k = c2A if ch % 2 == 0 else c2B
            a2 = a2A if (ch // 2) % 2 == 0 else a2B
            f0 = ch * F
            # ---- load 8 frames (4 parallel DMAs) ----
            xt = xp.tile([128, F, 441], bf16)
            for g in range(4):
                nc.sync.dma_start(
                    xt[:, 2 * g:2 * g + 2, :],
                    X[f0 + 2 * g:f0 + 2 * g + 2].rearrange("f p e -> p f e"))
            v = xt[:, :, :].rearrange("p f (i j) -> p f i j", i=21)
            # ---- conv1 direct to s2d layout ----
            for k in range(4):
                y0 = 1 if k < 2 else 0
                ps = ps1.tile([128, F, 5, 5], f32, tag="c1", name="ps")
                for rx in range(4):
                    x0 = 1 if rx < 2 else 0
                    J0 = 4 * x0 + rx - 2
                    for di in range(2):
                        I0 = 4 * y0 + k - 2 + di
                        rhs = v[:, :, I0:I0 + 17:4, J0:J0 + 17:4]
                        nc.tensor.matmul(
                            ps[32 * rx:32 * rx + 32, :, :, :],
                            w1sb[:, di, :], rhs,
                            start=(di == 0), stop=(di == 1),
                            tile_position=(0, 32 * rx))
                # epilogue: relu+bias, two halves (x0 differs by partition group)
                d0 = c2k[k][0:64, :, y0:y0 + 5, 1:6]
                d1 = c2k[k][64:128, :, y0:y0 + 5, 0:5]
                if k % 2 == 0:
                    nc.vector.tensor_scalar(d0, ps[0:64, :, :, :], b1sb[0:64],
                                            0.0, ALU.add, ALU.max)
                    nc.scalar.activation(d1, ps[64:128, :, :, :], AF.Relu,
                                         bias=b1sb[64:128])
                else:
                    nc.scalar.activation(d0, ps[0:64, :, :, :], AF.Relu,
                                         bias=b1sb[0:64])
                    nc.vector.tensor_scalar(d1, ps[64:128, :, :, :], b1sb[64:128],
                                            0.0, ALU.add, ALU.max)
            # ---- conv2 ----
            p2 = ps2.tile([64, F, 5, 5], f32)
            idx = 0
            for dq in range(4):
                di, dj = dq >> 1, dq & 1
                for k in range(4):
                    rhs = c2k[k][:, :, di:di + 5, dj:dj + 5]
                    nc.tensor.matmul(p2[:], w2sb[:, dq * 4 + k, :], rhs,
                                     start=(idx == 0), stop=(idx == 15))
                    idx += 1
            fo = (ch % 2) * F
            nc.scalar.activation(a2[:, fo:fo + F, 0:5, 0:5], p2[:], AF.Relu,
                                 bias=b2sb[:])
            # ---- conv3 every 2 chunks ----
            if ch % 2 == 1:
                p3 = ps3.tile([128, 16, 2, 2], f32)
                a2v = a2[:, :, :, :].rearrange(
                    "p f (Y s) (Xx t) -> p s t f Y Xx", s=2, t=2)
                idx = 0
                for ky in range(3):
                    for kx in range(3):
                        rhs = a2v[:, ky % 2, kx % 2, :, ky // 2:ky // 2 + 2,
                                  kx // 2:kx // 2 + 2]
                        nc.tensor.matmul(p3[:], w3sb[:, idx, :], rhs,
                                         start=(idx == 0), stop=(idx == 8))
                        idx += 1
                t0 = (ch // 2) * 16
                nc.scalar.activation(f3[:, t0:t0 + 16, :, :], p3[:],
                                     AF.Relu, bias=b3sb[:])

        # ---- fc1 (all T at once) ----
        cctx.close()
        psg = ctx.enter_context(tc.tile_pool(name="psg", bufs=4, space="PSUM"))
        f3v = f3[:, :, :, :].rearrange("p t a b -> p (a b) t")
        for oc in range(4):
            pf = psg.tile([128, T], f32, tag="g", name="pf")
            for j in range(4):
                nc.tensor.matmul(pf[:], fw1sb[:, j * 4 + oc, :], f3v[:, j, :],
                                 start=(j == 0), stop=(j == 3))
            nc.scalar.activation(fc1o[:, oc, :], pf[:], AF.Relu,
                                 bias=fb1sb[:, oc:oc + 1])
        # ---- gate input projections wx ----
        for gi in range(3):
            px = psg.tile([128, T], f32, tag="g", name="px")
            for oc in range(4):
                nc.tensor.matmul(px[:], wgsb[:, gi * 4 + oc, :], fc1o[:, oc, :],
                                 start=(oc == 0), stop=(oc == 3))
            nc.scalar.activation(wxsb[:, gi, :], px[:], AF.Identity,
                                 bias=gbsb[:, gi:gi + 1])

        # ---- DEER fixed-point GRU ----
        dp = ctx.enter_context(tc.tile_pool(name="deer", bufs=2))
        hs_final = None
        for s in range(NSWEEP):
            hp = hA if s % 2 == 0 else hB
            hn = hB if s % 2 == 0 else hA
            pz = psg.tile([128, T], f32, tag="g", name="pz")
            pr = psg.tile([128, T], f32, tag="g", name="pr")
            nc.tensor.matmul(pz[:], ugsb[:, 0, :], hp[:], start=True, stop=True)
            nc.tensor.matmul(pr[:], ugsb[:, 1, :], hp[:], start=True, stop=True)
            tz = dp.tile([128, T], f32, tag="tz")
            tr = dp.tile([128, T], f32, tag="tr")
            nc.vector.tensor_tensor(tz[:], pz[:], wxsb[:, 0, :], ALU.add)
            nc.vector.tensor_tensor(tr[:], pr[:], wxsb[:, 1, :], ALU.add)
            Z = dp.tile([128, T], f32, tag="Z")
            R = dp.tile([128, T], f32, tag="R")
            nc.scalar.activation(Z[:], tz[:], AF.Sigmoid)
            nc.scalar.activation(R[:], tr[:], AF.Sigmoid)
            RH = dp.tile([128, T], bf16, tag="RH")
            nc.vector.tensor_tensor(RH[:], R[:], hp[:], ALU.mult)
            ph = psg.tile([128, T], f32, tag="g", name="ph")
            nc.tensor.matmul(ph[:], ugsb[:, 2, :], RH[:], start=True, stop=True)
            th = dp.tile([128, T], f32, tag="th")
            nc.vector.tensor_tensor(th[:], ph[:], wxsb[:, 2, :], ALU.add)
            HH = dp.tile([128, T], f32, tag="HH")
            nc.scalar.activation(HH[:], th[:], AF.Tanh)
            A = dp.tile([128, T], f32, tag="A")
            nc.vector.tensor_scalar(A[:], Z[:], -1.0, 1.0, ALU.mult, ALU.add)
            Bt = dp.tile([128, T], f32, tag="B")
            nc.vector.tensor_tensor(Bt[:], Z[:], HH[:], ALU.mult)
            hs = dp.tile([128, T], bf16, tag="hs")
            nc.vector.tensor_tensor_scan(hs[:], A[:], Bt[:], 0.0,
                                         ALU.mult, ALU.add)
            if s < NSWEEP - 1:
                nc.vector.tensor_copy(hn[:, 1:T], hs[:, 0:T - 1])
            hs_final = hs

        # ---- fc3 + outputs ----
        po = psg.tile([18, T], f32, tag="g", name="po")
        nc.tensor.matmul(po[:], fw3sb[:], hs_final[:], start=True, stop=True)
        osb = dp.tile([18, T], f32, tag="osb")
        nc.scalar.activation(osb[:], po[:], AF.Identity, bias=fb3sb[:])
        nc.sync.dma_start(OUT[:, :].rearrange("t a -> a t"), osb[:])
        hl32 = dp.tile([128, 1], f32, tag="hl32")
        nc.vector.tensor_copy(hl32[:], hs_final[:, T - 1:T])
        nc.sync.dma_start(HL[:, :], hl32[:])

    nc.compile()
    return nc


def _prep_shared(inputs):
    """Host-side weight preprocessing (shared across cores)."""
    g = {k: np.asarray(v, dtype=np.float32) for k, v in inputs.items()}
    d = {}
    # conv1: lhsT halves [128, 2, 32]; p<64: (r,c); p>=64: (r, 8+c); per di
    w1 = g["conv1_w"]  # [32,1,16,16]
    w1s = np.zeros((128, 2, 32), np.float32)
    for di in range(2):
        for p in range(64):
            r, c = p // 8, p % 8
            w1s[p, di] = w1[:, 0, di * 8 + r, c]
            w1s[64 + p, di] = w1[:, 0, di * 8 + r, 8 + c]
    d["w1"] = _bf16(w1s)
    d["b1"] = np.tile(g["conv1_b"], 4).reshape(128, 1)
    # conv2 chunks: (dq=(di,dj), k): [p=32rx+c, o] = w2[o, c, 4di+k, 4dj+rx]
    w2 = g["conv2_w"]  # [64,32,8,8]
    w2s = np.zeros((128, 16, 64), np.float32)
    for dq in range(4):
        di, dj = dq >> 1, dq & 1
        for k in range(4):
            for rx in range(4):
                for c in range(32):
                    w2s[32 * rx + c, dq * 4 + k] = w2[:, c, 4 * di + k,
                                                      4 * dj + rx]
    d["w2"] = _bf16(w2s)
    d["b2"] = g["conv2_b"].reshape(64, 1)
    # conv3: [64, 9, 128]: lhsT = w3[:,:,ky,kx].T
    w3 = g["conv3_w"]  # [128,64,3,3]
    w3s = np.zeros((64, 9, 128), np.float32)
    for ky in range(3):
        for kx in range(3):
            w3s[:, ky * 3 + kx] = w3[:, :, ky, kx].T
    d["w3"] = _bf16(w3s)
    d["b3"] = g["conv3_b"].reshape(128, 1)
    # fc1 chunks (j, oc): fc1_w[oc*128:(oc+1)*128, j::4].T  (feat idx = c*4 + pos)
    fw = g["fc1_w"]  # [512, 512]
    fw1 = np.zeros((128, 16, 128), np.float32)
    for j in range(4):
        for oc in range(4):
            fw1[:, j * 4 + oc] = fw[oc * 128:(oc + 1) * 128, j::4].T
    d["fw1"] = _bf16(fw1)
    d["fb1"] = g["fc1_b"].reshape(128, 4, order="F").copy()
    # gates W chunks: (gate, oc): W[:, oc*128:(oc+1)*128].T over fc1o oc blocks
    wg = np.zeros((128, 12, 128), np.float32)
    for gi, Wm in enumerate([g["Wz"], g["Wr"], g["Wh"]]):
        for oc in range(4):
            wg[:, gi * 4 + oc] = Wm[:, oc * 128:(oc + 1) * 128].T
    d["wg"] = _bf16(wg)
    d["gb"] = np.stack([g["bz"], g["br"], g["bh"]], axis=1)
    ug = np.stack([g["Uz"].T, g["Ur"].T, g["Uh"].T], axis=1)
    d["ug"] = _bf16(ug)
    d["fw3"] = _bf16(g["fc3_w"].T)  # [128, 18]
    d["fb3"] = g["fc3_b"].reshape(18, 1)
    return d


def _prep_x(xb):
    """[T,160,160] f32 -> s2d bf16 [T, 128, 441] (half2 = J+1 shifted)."""
    xpad = np.zeros((T, 168, 176), np.float32)
    xpad[:, 4:164, 4:164] = xb
    blk = xpad.reshape(T, 21, 8, 22, 8)
    h1 = blk[:, :, :, 0:21].transpose(0, 2, 4, 1, 3).reshape(T, 64, 441)
    h2 = blk[:, :, :, 1:22].transpose(0, 2, 4, 1, 3).reshape(T, 64, 441)
    return _bf16(np.concatenate([h1, h2], axis=1))


def kernel(**inputs):
    from concourse.bass_utils import run_bass_kernel_spmd

    if "nc" not in _cache:
        _cache["nc"] = _build_graph()
    nc = _cache["nc"]

    shared = _prep_shared(inputs)
    x = np.asarray(inputs["x"], dtype=np.float32)  # [8,128,160,160,1]
    in_maps = []
    for b in range(BS):
        m = dict(shared)
        m["x"] = _prep_x(x[b, :, :, :, 0])
        in_maps.append(m)

    import os
    trace = bool(os.environ.get("BASS_KERNEL_TRACE"))
    if trace:
        import types
        if "antenv.axon_hooks" not in sys.modules:
            import antenv
            mod = types.ModuleType("antenv.axon_hooks")
            holder = {"hook": None}
            mod.set_axon_ntff_profile_hook = \
                lambda h: holder.__setitem__("hook", h)
            mod.get_axon_ntff_profile_hook = lambda: holder["hook"]
            sys.modules["antenv.axon_hooks"] = mod
            antenv.axon_hooks = mod
            try:
                from trn_agent_boot.trn_boot import _ntff_profile_via_ctypes
                mod.set_axon_ntff_profile_hook(
                    _ntff_profile_via_ctypes("/opt/axon/libaxon_pjrt.so"))
            except Exception as e:
                print("ntff hook registration failed:", e)
    res = run_bass_kernel_spmd(nc, in_maps, core_ids=list(range(BS)),
                               trace=trace)
    if trace:
        _cache["exec_time_ns"] = res.exec_time_ns
        _cache["profile_json"] = res.profile_json
    outs = res.results
    out = np.stack([outs[b]["out"] for b in range(BS)], axis=0)
    hl = np.stack([outs[b]["hlast"][:, 0] for b in range(BS)], axis=0)
    return out.astype(np.float32), hl.astype(np.float32)[None]


# revision 11
# speedup vs baseline: 3.0025x; 1.1639x over previous
import sys

sys.path.insert(0, "/opt/trn_rl_repo")

import numpy as np
import ml_dtypes
from contextlib import ExitStack

T, HID, FEAT, NACT = 128, 128, 512, 18
BS = 8
F = 16          # frames per pipeline chunk
NCH = T // F    # 8 chunks
NSWEEP = 6     # DEER fixed-point sweeps

_cache = {}


def _bf16(a):
    return np.asarray(a, dtype=np.float32).astype(ml_dtypes.bfloat16)


def _build_graph():
    from concourse import bacc, mybir, tile

    f32, bf16 = mybir.dt.float32, mybir.dt.bfloat16
    AF = mybir.ActivationFunctionType
    ALU = mybir.AluOpType

    nc = bacc.Bacc(None, target_bir_lowering=False)

    def din(name, shape, dt):
        return nc.declare_dram_parameter(name, shape, dt, isOutput=False)

    X = din("x", [T, 128, 441], bf16)
    W1 = din("w1", [128, 2, 32], bf16)
    B1 = din("b1", [128, 1], f32)
    W2 = din("w2", [128, 16, 64], bf16)
    B2 = din("b2", [64, 1], f32)
    W3 = din("w3", [64, 9, 128], bf16)
    B3 = din("b3", [128, 1], f32)
    FW1 = din("fw1", [128, 16, 128], bf16)   # (j*4+oc)
    FB1 = din("fb1", [128, 4], f32)
    WG = din("wg", [128, 12, 128], bf16)     # (gate*4+oc) lhsT chunks
    GB = din("gb", [128, 3], f32)
    UG = din("ug", [128, 3, 128], bf16)      # gate: Uz^T, Ur^T, Uh^T
    FW3 = din("fw3", [128, 18], bf16)
    FB3 = din("fb3", [18, 1], f32)
    OUT = nc.declare_dram_parameter("out", [NACT, T], f32, isOutput=True)
    HL = nc.declare_dram_parameter("hlast", [HID, 1], f32, isOutput=True)

    with tile.TileContext(nc) as tc, ExitStack() as ctx:
        consts = ctx.enter_context(tc.tile_pool(name="consts", bufs=1))

        w1sb = consts.tile([128, 2, 32], bf16)
        b1sb = consts.tile([128, 1], f32)
        w2sb = consts.tile([128, 16, 64], bf16)
        b2sb = consts.tile([64, 1], f32)
        w3sb = consts.tile([64, 9, 128], bf16)
        b3sb = consts.tile([128, 1], f32)
        fw1sb = consts.tile([128, 16, 128], bf16)
        fb1sb = consts.tile([128, 4], f32)
        wgsb = consts.tile([128, 12, 128], bf16)
        gbsb = consts.tile([128, 3], f32)
        ugsb = consts.tile([128, 3, 128], bf16)
        fw3sb = consts.tile([128, 18], bf16)
        fb3sb = consts.tile([18, 1], f32)
        for t_, d_ in [(w1sb, W1), (b1sb, B1), (w2sb, W2), (b2sb, B2),
                       (w3sb, W3), (b3sb, B3)]:
            nc.sync.dma_start(t_[:], d_[:])

        # persistent activations
        c2A = [consts.tile([128, F, 6, 6], bf16, name=f"c2A{k}", tag=f"c2A{k}")
               for k in range(4)]
        c2B = [consts.tile([128, F, 6, 6], bf16, name=f"c2B{k}", tag=f"c2B{k}")
               for k in range(4)]

        f3 = consts.tile([128, T, 2, 2], bf16)
        fc1o = consts.tile([128, 4, T], bf16)
        wxsb = consts.tile([128, 3, T], f32)
        hA = consts.tile([128, T], bf16)
        hB = consts.tile([128, T], bf16)

        for t_ in c2A + c2B:
            nc.vector.memset(t_[:], 0.0)
        nc.vector.memset(hA[:], 0.0)
        nc.vector.memset(hB[:], 0.0)

        cctx = ExitStack()
        xp = cctx.enter_context(tc.tile_pool(name="xin", bufs=3))
        a2p = cctx.enter_context(tc.tile_pool(name="a2", bufs=2))
        ps1 = cctx.enter_context(tc.tile_pool(name="ps1", bufs=4, space="PSUM"))
        ps2 = cctx.enter_context(tc.tile_pool(name="ps2", bufs=2, space="PSUM"))
        ps3 = cctx.enter_context(tc.tile_pool(name="ps3", bufs=2, space="PSUM"))

        for ch in range(NCH):
            c2k = c2A if ch % 2 == 0 else c2B
            f0 = ch * F
            # ---- load 16 frames (8 parallel DMAs) ----
            xt = xp.tile([128, F, 441], bf16)
            for g in range(8):
                nc.sync.dma_start(
                    xt[:, 2 * g:2 * g + 2, :],
                    X[f0 + 2 * g:f0 + 2 * g + 2].rearrange("f p e -> p f e"))
            v = xt[:, :, :].rearrange("p f (i j) -> p f i j", i=21)
            # ---- conv1 direct to s2d layout ----
            for k in range(4):
                y0 = 1 if k < 2 else 0
                ps = ps1.tile([128, F, 5, 5], f32, tag="c1", name="ps")
                for rx in range(4):
                    x0 = 1 if rx < 2 else 0
                    J0 = 4 * x0 + rx - 2
                    for di in range(2):
                        I0 = 4 * y0 + k - 2 + di
                        rhs = v[:, :, I0:I0 + 17:4, J0:J0 + 17:4]
                        nc.tensor.matmul(
                            ps[32 * rx:32 * rx + 32, :, :, :],
                            w1sb[:, di, :], rhs,
                            start=(di == 0), stop=(di == 1),
                            tile_position=(0, 32 * rx))
                # epilogue: relu+bias, two halves (x0 differs by partition group)
                d0 = c2k[k][0:64, :, y0:y0 + 5, 1:6]
                d1 = c2k[k][64:128, :, y0:y0 + 5, 0:5]
                if k % 2 == 0:
                    nc.vector.tensor_scalar(d0, ps[0:64, :, :, :], b1sb[0:64],
                                            0.0, ALU.add, ALU.max)
                    nc.scalar.activation(d1, ps[64:128, :, :, :], AF.Relu,
                                         bias=b1sb[64:128])
                else:
                    nc.scalar.activation(d0, ps[0:64, :, :, :], AF.Relu,
                                         bias=b1sb[0:64])
                    nc.vector.tensor_scalar(d1, ps[64:128, :, :, :], b1sb[64:128],
                                            0.0, ALU.add, ALU.max)
            # ---- conv2 ----
            p2 = ps2.tile([64, F, 5, 5], f32)
            idx = 0
            for dq in range(4):
                di, dj = dq >> 1, dq & 1
                for k in range(4):
                    rhs = c2k[k][:, :, di:di + 5, dj:dj + 5]
                    nc.tensor.matmul(p2[:], w2sb[:, dq * 4 + k, :], rhs,
                                     start=(idx == 0), stop=(idx == 15))
                    idx += 1
            a2 = a2p.tile([64, F, 6, 6], bf16)
            nc.scalar.activation(a2[:, :, 0:5, 0:5], p2[:], AF.Relu,
                                 bias=b2sb[:])
            # ---- conv3 ----
            p3 = ps3.tile([128, F, 2, 2], f32)
            a2v = a2[:, :, :, :].rearrange(
                "p f (Y s) (Xx t) -> p s t f Y Xx", s=2, t=2)
            idx = 0
            for ky in range(3):
                for kx in range(3):
                    rhs = a2v[:, ky % 2, kx % 2, :, ky // 2:ky // 2 + 2,
                              kx // 2:kx // 2 + 2]
                    nc.tensor.matmul(p3[:], w3sb[:, idx, :], rhs,
                                     start=(idx == 0), stop=(idx == 8))
                    idx += 1
            t0 = ch * F
            nc.scalar.activation(f3[:, t0:t0 + F, :, :], p3[:],
                                 AF.Relu, bias=b3sb[:])

        # ---- late const loads (deferred off the conv critical path) ----
        for t_, d_ in [(fw1sb, FW1), (fb1sb, FB1), (wgsb, WG), (gbsb, GB),
                       (ugsb, UG), (fw3sb, FW3), (fb3sb, FB3)]:
            nc.sync.dma_start(t_[:], d_[:])
        # ---- fc1 (all T at once) ----
        cctx.close()
        psg = ctx.enter_context(tc.tile_pool(name="psg", bufs=4, space="PSUM"))
        f3v = f3[:, :, :, :].rearrange("p t a b -> p (a b) t")
        for oc in range(4):
            pf = psg.tile([128, T], f32, tag="g", name="pf")
            for j in range(4):
                nc.tensor.matmul(pf[:], fw1sb[:, j * 4 + oc, :], f3v[:, j, :],
                                 start=(j == 0), stop=(j == 3))
            nc.scalar.activation(fc1o[:, oc, :], pf[:], AF.Relu,
                                 bias=fb1sb[:, oc:oc + 1])
        # ---- gate input projections wx ----
        for gi in range(3):
            px = psg.tile([128, T], f32, tag="g", name="px")
            for oc in range(4):
                nc.tensor.matmul(px[:], wgsb[:, gi * 4 + oc, :], fc1o[:, oc, :],
                                 start=(oc == 0), stop=(oc == 3))
            nc.scalar.activation(wxsb[:, gi, :], px[:], AF.Identity,
                                 bias=gbsb[:, gi:gi + 1])

        # ---- DEER fixed-point GRU ----
        dp = ctx.enter_context(tc.tile_pool(name="deer", bufs=2))
        hs_final = None
        for s in range(NSWEEP):
            hp = hA if s % 2 == 0 else hB
            hn = hB if s % 2 == 0 else hA
            pz = psg.tile([128, T], f32, tag="g", name="pz")
            pr = psg.tile([128, T], f32, tag="g", name="pr")
            nc.scalar.activation(pz[:], wxsb[:, 0, :], AF.Identity)
            nc.vector.tensor_copy(pr[:], wxsb[:, 1, :])
            nc.tensor.matmul(pz[:], ugsb[:, 0, :], hp[:], start=False, stop=True,
                             skip_group_check=True)
            nc.tensor.matmul(pr[:], ugsb[:, 1, :], hp[:], start=False, stop=True,
                             skip_group_check=True)
            Z = dp.tile([128, T], f32, tag="Z")
            A = dp.tile([128, T], f32, tag="A")
            R = dp.tile([128, T], f32, tag="R")
            nc.scalar.activation(R[:], pr[:], AF.Sigmoid)
            nc.scalar.activation(Z[:], pz[:], AF.Sigmoid)
            nc.scalar.activation(A[:], pz[:], AF.Sigmoid, scale=-1.0)
            RH = dp.tile([128, T], bf16, tag="RH")
            nc.vector.tensor_tensor(RH[:], R[:], hp[:], ALU.mult)
            ph = psg.tile([128, T], f32, tag="g", name="ph")
            nc.vector.tensor_copy(ph[:], wxsb[:, 2, :])
            nc.tensor.matmul(ph[:], ugsb[:, 2, :], RH[:], start=False, stop=True,
                             skip_group_check=True)
            HH = dp.tile([128, T], f32, tag="HH")
            nc.scalar.activation(HH[:], ph[:], AF.Tanh)
            Bt = dp.tile([128, T], f32, tag="B")
            nc.vector.tensor_tensor(Bt[:], Z[:], HH[:], ALU.mult)
            hs = dp.tile([128, T], bf16, tag="hs")
            nc.vector.tensor_tensor_scan(hs[:], A[:], Bt[:], 0.0,
                                         ALU.mult, ALU.add)
            if s < NSWEEP - 1:
                nc.vector.tensor_copy(hn[:, 1:T], hs[:, 0:T - 1])
            hs_final = hs

        # ---- fc3 + outputs ----
        po = psg.tile([18, T], f32, tag="g", name="po")
        nc.tensor.matmul(po[:], fw3sb[:], hs_final[:], start=True, stop=True)
        osb = dp.tile([18, T], f32, tag="osb")
        nc.scalar.activation(osb[:], po[:], AF.Identity, bias=fb3sb[:])
        nc.sync.dma_start(OUT[:, :], osb[:])
        hl32 = dp.tile([128, 1], f32, tag="hl32")
        nc.vector.tensor_copy(hl32[:], hs_final[:, T - 1:T])
        nc.sync.dma_start(HL[:, :], hl32[:])

    nc.compile()
    return nc


def _prep_shared(inputs):
    """Host-side weight preprocessing (shared across cores)."""
    g = {k: np.asarray(v, dtype=np.float32) for k, v in inputs.items()}
    d = {}
    # conv1: lhsT halves [128, 2, 32]; p<64: (r,c); p>=64: (r, 8+c); per di
    w1 = g["conv1_w"]  # [32,1,16,16]
    w1s = np.zeros((128, 2, 32), np.float32)
    for di in range(2):
        for p in range(64):
            r, c = p // 8, p % 8
            w1s[p, di] = w1[:, 0, di * 8 + r, c]
            w1s[64 + p, di] = w1[:, 0, di * 8 + r, 8 + c]
    d["w1"] = _bf16(w1s)
    d["b1"] = np.tile(g["conv1_b"], 4).reshape(128, 1)
    # conv2 chunks: (dq=(di,dj), k): [p=32rx+c, o] = w2[o, c, 4di+k, 4dj+rx]
    w2 = g["conv2_w"]  # [64,32,8,8]
    w2s = np.zeros((128, 16, 64), np.float32)
    for dq in range(4):
        di, dj = dq >> 1, dq & 1
        for k in range(4):
            for rx in range(4):
                for c in range(32):
                    w2s[32 * rx + c, dq * 4 + k] = w2[:, c, 4 * di + k,
                                                      4 * dj + rx]
    d["w2"] = _bf16(w2s)
    d["b2"] = g["conv2_b"].reshape(64, 1)
    # conv3: [64, 9, 128]: lhsT = w3[:,:,ky,kx].T
    w3 = g["conv3_w"]  # [128,64,3,3]
    w3s = np.zeros((64, 9, 128), np.float32)
    for ky in range(3):
        for kx in range(3):
            w3s[:, ky * 3 + kx] = w3[:, :, ky, kx].T
    d["w3"] = _bf16(w3s)
    d["b3"] = g["conv3_b"].reshape(128, 1)
    # fc1 chunks (j, oc): fc1_w[oc*128:(oc+1)*128, j::4].T  (feat idx = c*4 + pos)
    fw = g["fc1_w"]  # [512, 512]
    fw1 = np.zeros((128, 16, 128), np.float32)
    for j in range(4):
        for oc in range(4):
            fw1[:, j * 4 + oc] = fw[oc * 128:(oc + 1) * 128, j::4].T
    d["fw1"] = _bf16(fw1)
    d["fb1"] = g["fc1_b"].reshape(128, 4, order="F").copy()
    # gates W chunks: (gate, oc): W[:, oc*128:(oc+1)*128].T over fc1o oc blocks
    wg = np.zeros((128, 12, 128), np.float32)
    for gi, Wm in enumerate([g["Wz"], g["Wr"], g["Wh"]]):
        for oc in range(4):
            wg[:, gi * 4 + oc] = Wm[:, oc * 128:(oc + 1) * 128].T
    d["wg"] = _bf16(wg)
    d["gb"] = np.stack([g["bz"], g["br"], g["bh"]], axis=1)
    ug = np.stack([g["Uz"].T, g["Ur"].T, g["Uh"].T], axis=1)
    d["ug"] = _bf16(ug)
    d["fw3"] = _bf16(g["fc3_w"].T)  # [128, 18]
    d["fb3"] = g["fc3_b"].reshape(18, 1)
    return d


def _prep_x(xb):
    """[T,160,160] f32 -> s2d bf16 [T, 128, 441] (half2 = J+1 shifted)."""
    xpad = np.zeros((T, 168, 176), np.float32)
    xpad[:, 4:164, 4:164] = xb
    blk = xpad.reshape(T, 21, 8, 22, 8)
    h1 = blk[:, :, :, 0:21].transpose(0, 2, 4, 1, 3).reshape(T, 64, 441)
    h2 = blk[:, :, :, 1:22].transpose(0, 2, 4, 1, 3).reshape(T, 64, 441)
    return _bf16(np.concatenate([h1, h2], axis=1))


def kernel(**inputs):
    from concourse.bass_utils import run_bass_kernel_spmd

    if "nc" not in _cache:
        _cache["nc"] = _build_graph()
    nc = _cache["nc"]

    shared = _prep_shared(inputs)
    x = np.asarray(inputs["x"], dtype=np.float32)  # [8,128,160,160,1]
    in_maps = []
    for b in range(BS):
        m = dict(shared)
        m["x"] = _prep_x(x[b, :, :, :, 0])
        in_maps.append(m)

    import os
    trace = bool(os.environ.get("BASS_KERNEL_TRACE"))
    if trace:
        import types
        if "antenv.axon_hooks" not in sys.modules:
            import antenv
            mod = types.ModuleType("antenv.axon_hooks")
            holder = {"hook": None}
            mod.set_axon_ntff_profile_hook = \
                lambda h: holder.__setitem__("hook", h)
            mod.get_axon_ntff_profile_hook = lambda: holder["hook"]
            sys.modules["antenv.axon_hooks"] = mod
            antenv.axon_hooks = mod
            try:
                from trn_agent_boot.trn_boot import _ntff_profile_via_ctypes
                mod.set_axon_ntff_profile_hook(
                    _ntff_profile_via_ctypes("/opt/axon/libaxon_pjrt.so"))
            except Exception as e:
                print("ntff hook registration failed:", e)
    res = run_bass_kernel_spmd(nc, in_maps, core_ids=list(range(BS)),
                               trace=trace)
    if trace:
        _cache["exec_time_ns"] = res.exec_time_ns
        _cache["profile_json"] = res.profile_json
    outs = res.results
    out = np.stack([outs[b]["out"].T for b in range(BS)], axis=0)
    hl = np.stack([outs[b]["hlast"][:, 0] for b in range(BS)], axis=0)
    return out.astype(np.float32), hl.astype(np.float32)[None]
